# revision 28
# baseline (speedup 1.0000x reference)
"""Trainium2 Bass kernel for nn_CentersDistance (retrieval_knn).

logits[k, n] = -||centers[k] - inputs[n]||^2
             = 2*(centers @ inputs.T)[k, n] - ||centers[k]||^2 - ||inputs[n]||^2

Strategy (8 NeuronCores, data-parallel over the N=8192 inputs):
  * device computes ONLY the cross term 2*c.x as an fp8(e4m3) DoubleRow
    matmul (the PE virtualizes to 128x256 with 2 fp8 weights/cell:
    [256k,128m,512n] per matmul, measured 216ns warm = the 157 TF/s peak),
    accumulated in fp32 PSUM, stored to HBM as fp16.
  * the norm terms -||c||^2 - ||x||^2 are precomputed on host in float64
    and added to the fp16 cross on host (0.05% of the FLOPs; removes the
    ncsq/nxsq loads and turns the DVE epilogue into a plain cast-copy).
  * DoubleRow operand APs are [128, 2, free] plane-major (walrus requires
    the pair dim at AP position 1 with stride%16==0; pair-interleaved
    layouts are rejected).  xt additionally splits the moving dim in two
    h-halves at the DRAM/SBUF layout level ([P, h, i, n], 1KB contiguous
    lines) so the first matmul gates on ct0+xt0h0 = 384KB instead of 512KB.
  * each weight tile ct[t][m] serves both h-groups back-to-back; the second
    matmul sets InstMatmult.ldweights=False to reuse the loaded array,
    halving LDWEIGHTS (which in DoubleRow costs +72% vs bf16).  This is
    what closed the gap from 427ns to 216ns per matmul together with:
  * N_WU bf16 warmup matmuls on an uninitialized scratch tile bridging the
    whole preamble-to-first-tile window: a PE idle gap lets the HAM clock
    gate re-throttle to half rate (427ns/mm) for ~1us after restart.
  * loads: the PE-gating chunks (ct0, xt0h0) go first on the Sync HW-DGE
    ring (it ramps ~1.1us earlier than the Act ring after the preamble
    barrier); xt0h1 leads the Act ring.  The load phase saturates the
    8-core aggregate HBM bandwidth (~200GB/s per ring observed), so the
    rest is split evenly: Sync carries tiles 2,3; Act carries tile 1.
  * pass 1 (groups 0-7, banks 0-7) runs the d-pair loop outermost so
    matmuls pace with the streaming loads; pass 2 (groups 8-15) runs d
    innermost (paired h-groups) so each output group retires early.
  * PSUM->SBUF drain: plain dtype-converting copies (fp32 PSUM -> fp16
    SBUF) alternate between the DVE (tensor_scalar_add 0.0, even groups)
    and Act (activation Copy, odd groups) engines -- GPSIMD/Pool cannot
    access PSUM.  Only SP/Act have HW-DGE queues, so the Act engine issues
    its own groups' stores right after each copy (engines are in-order, so
    the data is ready) and the Sync engine stores the DVE's groups behind
    cp_sem_v.  The final group is drained in halves by both engines and
    stored on both rings to shorten the tail.  The PE's pass-2 bank-reuse
    wait is per-copy-engine (P10: concurrent PE-write + DVE-read of one
    PSUM bank is fatal).

History: bf16 exact-epilogue 45.1us (kernel_bf16_baseline.py); first fp8
DoubleRow 32.6us; +HAM bridge+weight reuse 30.6us; this version trims the
first-matmul gate and the drain tail.
"""

import threading
from contextlib import ExitStack

import numpy as np
import ml_dtypes

import concourse.mybir as mybir
from concourse import bacc
from concourse.bass_utils import run_bass_kernel_spmd

N_CORES = 8
N, K, D = 8192, 1024, 1024
NSH = N // N_CORES  # per-core slab of inputs
P = 128             # SBUF partitions
NF = 512            # matmul moving free dim (one fp32 PSUM bank)
T = 4               # DoubleRow contraction tiles (256 d-rows each)
HNF = NF // 2

M_TILES = K // P    # 8 center tiles
H_TILES = NSH // NF # 2 moving-dim tiles
G = M_TILES * H_TILES  # 16 output groups of [128, 512]
N_WU = 10           # PE warm-up matmuls (bridge preamble -> first tiles)

_DT8 = mybir.dt.float8e4
_NP8 = ml_dtypes.float8_e4m3
_DT16 = mybir.dt.float16

_cache = threading.local()


def _g_mh(g):
    return g // H_TILES, g % H_TILES


def _build_nc():
    nc = bacc.Bacc(
        "TRN2", target_bir_lowering=False, debug=False, num_devices=N_CORES
    )
    ct = nc.dram_tensor("ct", [T, P, 2, K], _DT8, kind="ExternalInput").ap()
    # xt layout [t, p, h, i, n]: h-half = 1KB contiguous per partition
    xt = nc.dram_tensor(
        "xt", [T, P, H_TILES, 2, NF], _DT8, kind="ExternalInput"
    ).ap()
    out = nc.dram_tensor("out", [K, NSH], _DT16, kind="ExternalOutput").ap()

    out_r = out.rearrange("(m p) n -> m p n", p=P)
    DR = mybir.MatmulPerfMode.DoubleRow

    with (
        nc.sbuf_tensor("wu_sb", [P, NF], mybir.dt.bfloat16) as wu_sb,
        nc.sbuf_tensor("ot_sb", [P, G * NF], _DT16) as ot_sb,
        ExitStack() as stack,
        nc.semaphore("mm_sem") as mm_sem,
        nc.semaphore("cp_sem_v") as cp_sem_v,
        nc.semaphore("cp_sem_g") as cp_sem_g,
        nc.semaphore("st_v") as st_v,
        nc.semaphore("st_g") as st_g,
        nc.Block() as block,
    ):
        d_sems = [stack.enter_context(nc.semaphore(f"d_sem{t}")) for t in range(T)]
        ct_sb = [
            stack.enter_context(nc.sbuf_tensor(f"ct_sb{t}", [P, 2, K], _DT8))
            for t in range(T)
        ]
        xt_sb = [
            stack.enter_context(
                nc.sbuf_tensor(f"xt_sb{t}", [P, H_TILES, 2, NF], _DT8)
            )
            for t in range(T)
        ]
        ps = [
            stack.enter_context(nc.psum_tensor(f"ps{b}", [P, NF], mybir.dt.float32))
            for b in range(8)
        ]

        def _load_half(eng, t, lo, hi):
            # partition-half DMAs (2KB contiguous lines): each tile is
            # split across BOTH HW-DGE rings so tile t completes at
            # ~ramp + (t+1)*1.3us instead of each ring's second tile
            # landing a whole ring-serialization later.  d_sem threshold
            # equals the total inc (4 x 16) so completion order between
            # the four DMAs cannot fake readiness.
            eng.dma_start(xt_sb[t][lo:hi], xt[t][lo:hi]).then_inc(d_sems[t], 16)
            eng.dma_start(ct_sb[t][lo:hi], ct[t][lo:hi]).then_inc(d_sems[t], 16)

        def _mm(g, t, reuse_weights):
            m, h = _g_mh(g)
            mm = nc.tensor.matmul(
                ps[g % 8][:],
                ct_sb[t][:, :, m * P : (m + 1) * P],
                xt_sb[t][:, h],
                start=(t == 0),
                stop=(t == T - 1),
                perf_mode=DR,
            )
            if reuse_weights:
                mm.ins.ldweights = False
            return mm

        # per-tile partition split between the rings: the Sync ring ramps
        # ~1.2us earlier after the preamble barrier, so it carries 3/4 of
        # the PE-gating tile 0; tile 3 rebalances the totals (1MB each)
        _SPLIT = [96, 64, 64, 32]  # partitions of tile t on the Sync ring

        @block.sync
        def _(sync):
            for t in range(T):
                _load_half(sync, t, 0, _SPLIT[t])
            for idx, g in enumerate(range(0, G, 2)):
                m, h = _g_mh(g)
                sync.wait_ge(cp_sem_v, idx + 1)
                sync.dma_start(
                    out_r[m][:, h * NF : (h + 1) * NF],
                    ot_sb[:, g * NF : (g + 1) * NF],
                ).then_inc(st_v, 16)
            # first half of the split final group (DVE copy #9)
            m, h = _g_mh(G - 1)
            sync.wait_ge(cp_sem_v, G // 2 + 1)
            sync.dma_start(
                out_r[m][:, h * NF : h * NF + HNF],
                ot_sb[:, (G - 1) * NF : (G - 1) * NF + HNF],
            ).then_inc(st_v, 16)
            sync.wait_ge(st_v, (G // 2 + 1) * 16)

        @block.scalar
        def _(scalar):
            for t in range(T):
                _load_half(scalar, t, _SPLIT[t], P)
            for g in range(1, G - 1, 2):
                m, h = _g_mh(g)
                scalar.wait_ge(mm_sem, g + 1)
                nc.scalar.activation(
                    ot_sb[:, g * NF : (g + 1) * NF],
                    ps[g % 8][:],
                    mybir.ActivationFunctionType.Copy,
                ).then_inc(cp_sem_g, 1)
                scalar.dma_start(
                    out_r[m][:, h * NF : (h + 1) * NF],
                    ot_sb[:, g * NF : (g + 1) * NF],
                ).then_inc(st_g, 16)
            # second half of the split final group
            m, h = _g_mh(G - 1)
            scalar.wait_ge(mm_sem, G)
            nc.scalar.activation(
                ot_sb[:, (G - 1) * NF + HNF : G * NF],
                ps[(G - 1) % 8][:, HNF:],
                mybir.ActivationFunctionType.Copy,
            ).then_inc(cp_sem_g, 1)
            scalar.dma_start(
                out_r[m][:, h * NF + HNF : (h + 1) * NF],
                ot_sb[:, (G - 1) * NF + HNF : G * NF],
            ).then_inc(st_g, 16)
            scalar.wait_ge(st_g, (G // 2) * 16)

        @block.tensor
        def _(tensor):
            # warm-up: keep the HAM clock gate open from the preamble until
            # the first chunks land.  wu_sb is deliberately uninitialized -
            # the products are never read; bank 7 is rewritten with
            # start=True by group 7's first real matmul.
            for _ in range(N_WU):
                nc.tensor.matmul(
                    ps[7][:], wu_sb[:, 0:P], wu_sb[:], start=True, stop=True
                )
            # pass 1: groups 0-7 accumulate in banks 0-7, d-pair outermost
            # so matmuls pace with the streaming loads; the two h-groups of
            # each m share one weight load
            for t in range(T):
                tensor.wait_ge(d_sems[t], 64)
                for m in range(4):
                    for h in range(2):
                        g = 2 * m + h
                        mm = _mm(g, t, reuse_weights=(h == 1))
                        if t == T - 1:
                            mm.then_inc(mm_sem, 1)
            # pass 2: groups 8-15 reuse banks 0-7 once the copy engine has
            # drained the pass-1 group from that bank; h-pairs interleave
            # so each weight tile is loaded once
            for jp in range(4):
                ga, gb = 8 + 2 * jp, 9 + 2 * jp
                tensor.wait_ge(cp_sem_v, jp + 1)   # bank 2jp   (group 2jp)
                tensor.wait_ge(cp_sem_g, jp + 1)   # bank 2jp+1 (group 2jp+1)
                for t in range(T):
                    mma = _mm(ga, t, reuse_weights=False)
                    mmb = _mm(gb, t, reuse_weights=True)
                    if t == T - 1:
                        mma.then_inc(mm_sem, 1)
                        mmb.then_inc(mm_sem, 1)

        @block.vector
        def _(vector):
            for g in range(0, G, 2):
                vector.wait_ge(mm_sem, g + 1)
                vector.tensor_scalar_add(
                    ot_sb[:, g * NF : (g + 1) * NF], ps[g % 8][:], 0.0
                ).then_inc(cp_sem_v, 1)
            # first half of the split final group
            vector.wait_ge(mm_sem, G)
            vector.tensor_scalar_add(
                ot_sb[:, (G - 1) * NF : (G - 1) * NF + HNF],
                ps[(G - 1) % 8][:, :HNF],
                0.0,
            ).then_inc(cp_sem_v, 1)

    nc.compile()
    return nc


def _get_nc():
    if not hasattr(_cache, "nc"):
        _cache.nc = _build_nc()
    return _cache.nc


def kernel(inputs, centers, _trace=False):
    inputs = np.asarray(inputs, dtype=np.float32)
    centers = np.asarray(centers, dtype=np.float32)

    csq = np.sum(centers.astype(np.float64) ** 2, axis=1)  # (K,)
    xsq = np.sum(inputs.astype(np.float64) ** 2, axis=1)   # (N,)

    # DoubleRow layout: sub-row i covers d = t*256 + i*128 + p
    ct8 = np.ascontiguousarray(centers.T).astype(_NP8)      # [D, K]
    ct_dr = np.ascontiguousarray(
        ct8.reshape(T, 2, P, K).transpose(0, 2, 1, 3)       # [t, p, i, k]
    )
    xt8 = np.ascontiguousarray((2.0 * inputs).T).astype(_NP8)  # [D, N]
    xt_ti = xt8.reshape(T, 2, P, N)                         # [t, i, p, n]

    in_maps = []
    for i in range(N_CORES):
        sl = slice(i * NSH, (i + 1) * NSH)
        # [t, i, p, nsh] -> [t, p, h, i, n]
        core = xt_ti[:, :, :, sl].reshape(T, 2, P, H_TILES, NF)
        in_maps.append(
            {
                "ct": ct_dr,
                "xt": np.ascontiguousarray(core.transpose(0, 2, 3, 1, 4)),
            }
        )

    nc = _get_nc()
    try:
        res = run_bass_kernel_spmd(
            nc, in_maps, core_ids=list(range(N_CORES)), trace=_trace
        )
    except ModuleNotFoundError:
        # NTFF trace glue is absent in some images; rerun without tracing
        res = run_bass_kernel_spmd(
            nc, in_maps, core_ids=list(range(N_CORES)), trace=False
        )
    if _trace:
        kernel.last_results = res
    cross = np.concatenate([r["out"] for r in res.results], axis=1)  # fp16
    logits = cross.astype(np.float32)
    logits -= csq.astype(np.float32)[:, None]
    logits -= xsq.astype(np.float32)[None, :]
    return logits
